# revision 13
# baseline (speedup 1.0000x reference)
"""Multi-head self-attention Trainium2 kernel (8 NeuronCores, SPMD).

Sharding: data-parallel over the batch dim B=8 -> one batch per core.
Each core computes the full attention pipeline for its batch on-chip:

  qkvT = (x @ w_qkv)^T           (q,k transposed; v natural layout)
  sT_h[m,n] = k_h @ q_h^T        (keys on partitions, queries on free dim)
  expT = exp(sT)                 (scores in [-2,2] -> no max subtraction)
  outT_h = [v_h | 1]^T @ expT    (ones column yields softmax denominator)
  out_h = outT_h[:64] / outT_h[64]
  yT = w_proj^T @ outT + b_proj  (host transposes back)

All matmuls run in bf16 with fp32 PSUM accumulation. The 1/sqrt(dk)=1/8
scale is folded into w_q/b_q on the host (exact, power of two).
"""

from contextlib import ExitStack

import numpy as np
import ml_dtypes

import concourse.bass as bass
import concourse.mybir as mybir
import concourse.tile as tile
from concourse import bacc

BF16 = mybir.dt.bfloat16
F32 = mybir.dt.float32
P = 128  # SBUF partitions


def build_module(N=1024, D=1024, H=16, DK=64):
    """Build the single-core Bass module (same program runs SPMD on 8 cores)."""
    KC = D // P           # contraction chunks over feature dim
    MC = N // P           # token chunks
    FREE = min(512, N)    # moving free-dim per matmul (one PSUM bank of fp32)
    NF = N // FREE        # free-dim tiles per row of N
    HPC = P // DK         # heads per 128-row chunk (2)
    assert H == KC * HPC

    nc = bacc.Bacc("TRN2", target_bir_lowering=False, debug=False)

    xT_d = nc.dram_tensor("xT", [D, N], BF16, kind="ExternalInput").ap()
    wq_d = nc.dram_tensor("wq", [D, D], BF16, kind="ExternalInput").ap()
    wk_d = nc.dram_tensor("wk", [D, D], BF16, kind="ExternalInput").ap()
    wv_d = nc.dram_tensor("wv", [D, D], BF16, kind="ExternalInput").ap()
    wp_d = nc.dram_tensor("wp", [D, D], BF16, kind="ExternalInput").ap()
    bq_d = nc.dram_tensor("bq", [P, KC], F32, kind="ExternalInput").ap()
    bk_d = nc.dram_tensor("bk", [P, KC], F32, kind="ExternalInput").ap()
    bvb_d = nc.dram_tensor("bvb", [P, D], F32, kind="ExternalInput").ap()
    bp_d = nc.dram_tensor("bp", [P, KC], F32, kind="ExternalInput").ap()
    yT_d = nc.dram_tensor("yT", [D, N], F32, kind="ExternalOutput").ap()

    # DRAM views chunked to 128 partitions: row = c*128 + p
    xT_v = xT_d.rearrange("(c p) n -> p c n", p=P)
    wq_v = wq_d.rearrange("(c p) n -> p c n", p=P)
    wk_v = wk_d.rearrange("(c p) n -> p c n", p=P)
    wv_v = wv_d.rearrange("(c p) n -> p c n", p=P)
    wp_v = wp_d.rearrange("(c p) n -> p c n", p=P)
    yT_v = yT_d.rearrange("(c p) n -> p c n", p=P)

    with tile.TileContext(nc) as tc, ExitStack() as ctx:
        consts = ctx.enter_context(tc.tile_pool(name="consts", bufs=1))
        qkv_p = ctx.enter_context(tc.tile_pool(name="qkv", bufs=1))
        psA = ctx.enter_context(tc.tile_pool(name="psA", bufs=2, space="PSUM"))
        psB = ctx.enter_context(tc.tile_pool(name="psB", bufs=4, space="PSUM"))
        dram_p = ctx.enter_context(tc.tile_pool(name="dbounce", bufs=4, space="DRAM"))

        # ---- persistent tiles ----
        wq_sb = consts.tile([P, KC, D], BF16)
        wk_sb = consts.tile([P, KC, D], BF16)
        wv_sb = consts.tile([P, KC, D], BF16)
        wp_sb = consts.tile([P, KC, D], BF16)
        for c in range(KC):
            nc.sync.dma_start(out=wq_sb[:, c, :], in_=wq_v[:, c, :])
            nc.sync.dma_start(out=wk_sb[:, c, :], in_=wk_v[:, c, :])
            nc.sync.dma_start(out=wv_sb[:, c, :], in_=wv_v[:, c, :])
            nc.sync.dma_start(out=wp_sb[:, c, :], in_=wp_v[:, c, :])
        bq_sb = consts.tile([P, KC], F32)
        bk_sb = consts.tile([P, KC], F32)
        bp_sb = consts.tile([P, KC], F32)
        bvb_sb = consts.tile([P, D], F32)
        nc.sync.dma_start(out=bq_sb, in_=bq_d)
        nc.sync.dma_start(out=bk_sb, in_=bk_d)
        nc.sync.dma_start(out=bp_sb, in_=bp_d)
        nc.sync.dma_start(out=bvb_sb, in_=bvb_d)

        qT_sb = qkv_p.tile([P, KC, N], BF16)
        kT_sb = qkv_p.tile([P, KC, N], BF16)
        vaug_sb = qkv_p.tile([P, MC, H, DK + 1], BF16)
        nc.vector.memset(vaug_sb[:, :, :, DK : DK + 1], 1.0)
        oT_sb = qkv_p.tile([P, KC, N], BF16)

        # ---- qkv projections (xT lives in a scoped pool, freed after) ----
        with tc.tile_pool(name="xtp", bufs=1) as xt_pool:
            xT_sb = xt_pool.tile([P, KC, N], BF16)
            for c in range(KC):
                nc.sync.dma_start(out=xT_sb[:, c, :], in_=xT_v[:, c, :])
            # qT/kT: out[c*128+p, n] = sum_d w[d, c*128+p] * xT[d, n]
            for c in range(KC):
                for dst_sb, w_sb, b_sb in (
                    (qT_sb, wq_sb, bq_sb),
                    (kT_sb, wk_sb, bk_sb),
                ):
                    ps = psA.tile([P, N], F32, tag="psA")
                    for f in range(NF):
                        for k in range(KC):
                            nc.tensor.matmul(
                                ps[:, f * FREE : (f + 1) * FREE],
                                lhsT=w_sb[:, k, c * P : (c + 1) * P],
                                rhs=xT_sb[:, k, f * FREE : (f + 1) * FREE],
                                start=(k == 0),
                                stop=(k == KC - 1),
                            )
                    nc.scalar.activation(
                        out=dst_sb[:, c, :],
                        in_=ps,
                        func=mybir.ActivationFunctionType.Identity,
                        bias=b_sb[:, c : c + 1],
                    )
            # v natural: out[m, d] = sum_k x[m, k] w_v[k, d]  (lhsT = xT chunk)
            for c in range(MC):
                ps = psA.tile([P, D], F32, tag="psA")
                for f in range(D // FREE):
                    for k in range(KC):
                        nc.tensor.matmul(
                            ps[:, f * FREE : (f + 1) * FREE],
                            lhsT=xT_sb[:, k, c * P : (c + 1) * P],
                            rhs=wv_sb[:, k, f * FREE : (f + 1) * FREE],
                            start=(k == 0),
                            stop=(k == KC - 1),
                        )
                nc.vector.tensor_add(
                    out=vaug_sb[:, c, :, 0:DK],
                    in0=ps,
                    in1=bvb_sb,
                )

        exp_p = ctx.enter_context(tc.tile_pool(name="expp", bufs=2))
        misc_p = ctx.enter_context(tc.tile_pool(name="misc", bufs=2))
        ystage_p = ctx.enter_context(tc.tile_pool(name="ystage", bufs=2))

        # ---- attention (per pair of heads sharing a 128-row chunk) ----
        for c in range(KC):
            exA = exp_p.tile([P, MC, N], BF16, tag="exp")
            exB = exp_p.tile([P, MC, N], BF16, tag="exp")
            for j in range(MC):
                sA = psA.tile([P, N], F32, tag="psA")
                sB = psA.tile([P, N], F32, tag="psA")
                for f in range(NF):
                    nc.tensor.matmul(
                        sA[:, f * FREE : (f + 1) * FREE],
                        lhsT=kT_sb[0:DK, c, j * P : (j + 1) * P],
                        rhs=qT_sb[0:DK, c, f * FREE : (f + 1) * FREE],
                        start=True,
                        stop=True,
                        tile_position=(0, 0),
                    )
                    nc.tensor.matmul(
                        sB[:, f * FREE : (f + 1) * FREE],
                        lhsT=kT_sb[DK:P, c, j * P : (j + 1) * P],
                        rhs=qT_sb[DK:P, c, f * FREE : (f + 1) * FREE],
                        start=True,
                        stop=True,
                        tile_position=(DK, 0),
                    )
                nc.scalar.activation(
                    out=exA[:, j, :], in_=sA, func=mybir.ActivationFunctionType.Exp
                )
                nc.scalar.activation(
                    out=exB[:, j, :], in_=sB, func=mybir.ActivationFunctionType.Exp
                )
            for hl, ex in ((0, exA), (1, exB)):
                h = c * 2 + hl
                for f in range(NF):
                    po = psB.tile([DK + 1, FREE], F32, tag="psB")
                    for j in range(MC):
                        nc.tensor.matmul(
                            po,
                            lhsT=vaug_sb[:, j, h, :],
                            rhs=ex[:, j, f * FREE : (f + 1) * FREE],
                            start=(j == 0),
                            stop=(j == MC - 1),
                        )
                    rc = misc_p.tile([1, FREE], F32, tag="recip")
                    nc.vector.reciprocal(out=rc, in_=po[DK : DK + 1, :])
                    # broadcast 1/denom to DK partitions via a DRAM bounce
                    # (0-stride partition reads are only legal on DRAM APs)
                    dt_ = dram_p.tile([1, FREE], F32, tag="dbounce")
                    nc.sync.dma_start(out=dt_, in_=rc)
                    rcb = misc_p.tile([DK, FREE], F32, tag="rcb")
                    dt_bcast = bass.AP(
                        tensor=dt_.tensor,
                        offset=dt_.offset,
                        ap=[[0, DK]] + [list(d) for d in dt_.ap[1:]],
                    )
                    nc.sync.dma_start(out=rcb, in_=dt_bcast)
                    tmpo = misc_p.tile([DK, FREE], BF16, tag="tmpo")
                    nc.vector.tensor_mul(out=tmpo, in0=po[0:DK, :], in1=rcb)
                    nc.sync.dma_start(
                        out=oT_sb[
                            hl * DK : (hl + 1) * DK, c, f * FREE : (f + 1) * FREE
                        ],
                        in_=tmpo,
                    )

        # ---- output projection ----
        for c in range(KC):
            ps = psA.tile([P, N], F32, tag="psA")
            for f in range(NF):
                for k in range(KC):
                    nc.tensor.matmul(
                        ps[:, f * FREE : (f + 1) * FREE],
                        lhsT=wp_sb[:, k, c * P : (c + 1) * P],
                        rhs=oT_sb[:, k, f * FREE : (f + 1) * FREE],
                        start=(k == 0),
                        stop=(k == KC - 1),
                    )
            yst = ystage_p.tile([P, N], F32, tag="yst")
            nc.scalar.activation(
                out=yst,
                in_=ps,
                func=mybir.ActivationFunctionType.Identity,
                bias=bp_sb[:, c : c + 1],
            )
            nc.sync.dma_start(out=yT_v[:, c, :], in_=yst)

    nc.compile()
    return nc


def make_in_maps(x, w_qkv, b_qkv, w_proj, b_proj, N=1024, D=1024, H=16, DK=64):
    """Host-side prep: shard over batch, fold scale, transpose x, cast bf16."""
    bf = ml_dtypes.bfloat16
    KC = D // P
    scale = np.float32(1.0 / np.sqrt(DK))
    wq = np.ascontiguousarray((w_qkv[:, :D] * scale)).astype(bf)
    wk = np.ascontiguousarray(w_qkv[:, D : 2 * D]).astype(bf)
    wv = np.ascontiguousarray(w_qkv[:, 2 * D :]).astype(bf)
    wp = np.ascontiguousarray(w_proj).astype(bf)
    bq = np.ascontiguousarray((b_qkv[:D] * scale).reshape(KC, P).T).astype(np.float32)
    bk = np.ascontiguousarray(b_qkv[D : 2 * D].reshape(KC, P).T).astype(np.float32)
    bvb = np.ascontiguousarray(
        np.broadcast_to(b_qkv[2 * D :], (P, D))
    ).astype(np.float32)
    bp = np.ascontiguousarray(b_proj.reshape(KC, P).T).astype(np.float32)
    in_maps = []
    for b in range(x.shape[0]):
        xT = np.ascontiguousarray(x[b].T).astype(bf)
        in_maps.append(
            dict(xT=xT, wq=wq, wk=wk, wv=wv, wp=wp, bq=bq, bk=bk, bvb=bvb, bp=bp)
        )
    return in_maps


_module_cache = {}


def kernel(x, w_qkv, b_qkv, w_proj, b_proj):
    from concourse.bass_utils import run_bass_kernel_spmd

    x = np.asarray(x)
    B = x.shape[0]
    if "nc" not in _module_cache:
        _module_cache["nc"] = build_module()
    nc = _module_cache["nc"]
    in_maps = make_in_maps(
        x, np.asarray(w_qkv), np.asarray(b_qkv), np.asarray(w_proj), np.asarray(b_proj)
    )
    res = run_bass_kernel_spmd(nc, in_maps, core_ids=list(range(B)))
    out = np.stack([np.asarray(r["yT"]).T for r in res.results], axis=0)
    return np.ascontiguousarray(out.astype(np.float32))


# revision 20
# speedup vs baseline: 20.5754x; 20.5754x over previous
"""Multi-head self-attention Trainium2 kernel (8 NeuronCores, SPMD).

Sharding: data-parallel over the batch dim B=8 -> one batch per core.
Each core computes the full attention pipeline for its batch on-chip:

  qkvT = (x @ w_qkv)^T           (q,k transposed; v natural layout)
  sT_h[m,n] = k_h @ q_h^T        (keys on partitions, queries on free dim)
  expT = exp(sT)                 (scores in [-2,2] -> no max subtraction)
  outT_h = [v_h | 1]^T @ expT    (ones column yields softmax denominator)
  out_h = outT_h[:64] / outT_h[64]
  yT = w_proj^T @ outT + b_proj  (host transposes back)

All matmuls run in bf16 with fp32 PSUM accumulation. The 1/sqrt(dk)=1/8
scale is folded into w_q/b_q on the host (exact, power of two). Scores
for the two heads sharing a 128-row qkvT chunk run concurrently in
separate PE row groups (tile_position), and the ones-column of the
augmented V matrix makes the PE emit softmax denominators for free.
"""

from contextlib import ExitStack

import numpy as np
import ml_dtypes

import concourse.bass as bass
import concourse.mybir as mybir
import concourse.tile as tile
from concourse import bacc

BF16 = mybir.dt.bfloat16
F32 = mybir.dt.float32
P = 128  # SBUF partitions


def build_module(N=1024, D=1024, H=16, DK=64, reps=1):
    """Build the single-core Bass module (same program runs SPMD on 8 cores)."""
    KC = D // P           # contraction chunks over feature dim
    MC = N // P           # token chunks
    FREE = min(512, N)    # moving free-dim per matmul (one PSUM bank of fp32)
    NF = N // FREE        # free-dim tiles per row of N
    HPC = P // DK         # heads per 128-row chunk (2)
    assert H == KC * HPC

    nc = bacc.Bacc("TRN2", target_bir_lowering=False, debug=False)

    xT_d = nc.dram_tensor("xT", [D, N], BF16, kind="ExternalInput").ap()
    wq_d = nc.dram_tensor("wq", [D, D], BF16, kind="ExternalInput").ap()
    wk_d = nc.dram_tensor("wk", [D, D], BF16, kind="ExternalInput").ap()
    wv_d = nc.dram_tensor("wv", [D, D], BF16, kind="ExternalInput").ap()
    wp_d = nc.dram_tensor("wp", [D, D], BF16, kind="ExternalInput").ap()
    bq_d = nc.dram_tensor("bq", [P, KC], F32, kind="ExternalInput").ap()
    bk_d = nc.dram_tensor("bk", [P, KC], F32, kind="ExternalInput").ap()
    bvb_d = nc.dram_tensor("bvb", [P, D], BF16, kind="ExternalInput").ap()
    bp_d = nc.dram_tensor("bp", [P, KC], F32, kind="ExternalInput").ap()
    yT_d = nc.dram_tensor("yT", [D, N], F32, kind="ExternalOutput").ap()

    # DRAM views chunked to 128 partitions: row = c*128 + p
    xT_v = xT_d.rearrange("(c p) n -> p c n", p=P)
    wq_v = wq_d.rearrange("(c p) n -> p c n", p=P)
    wk_v = wk_d.rearrange("(c p) n -> p c n", p=P)
    wv_v = wv_d.rearrange("(c p) n -> p c n", p=P)
    wp_v = wp_d.rearrange("(c p) n -> p c n", p=P)
    yT_v = yT_d.rearrange("(c p) n -> p c n", p=P)

    with tile.TileContext(nc) as tc, ExitStack() as ctx:
        consts = ctx.enter_context(tc.tile_pool(name="consts", bufs=1))
        qkv_p = ctx.enter_context(tc.tile_pool(name="qkv", bufs=1))
        psA = ctx.enter_context(tc.tile_pool(name="psA", bufs=2, space="PSUM"))
        psB = ctx.enter_context(tc.tile_pool(name="psB", bufs=4, space="PSUM"))
        dram_p = ctx.enter_context(tc.tile_pool(name="dbounce", bufs=6, space="DRAM"))

        # ---- persistent tiles / input loads ----
        wq_sb = consts.tile([P, KC, D], BF16)
        wk_sb = consts.tile([P, KC, D], BF16)
        wv_sb = consts.tile([P, KC, D], BF16)
        wp_sb = consts.tile([P, KC, D], BF16)
        for c in range(KC):
            nc.sync.dma_start(out=wq_sb[:, c, :], in_=wq_v[:, c, :])
            nc.sync.dma_start(out=wk_sb[:, c, :], in_=wk_v[:, c, :])
            nc.sync.dma_start(out=wv_sb[:, c, :], in_=wv_v[:, c, :])
            nc.sync.dma_start(out=wp_sb[:, c, :], in_=wp_v[:, c, :])
        bq_sb = consts.tile([P, KC], F32)
        bk_sb = consts.tile([P, KC], F32)
        bp_sb = consts.tile([P, KC], F32)
        bvb_sb = consts.tile([P, D], BF16)
        nc.sync.dma_start(out=bq_sb, in_=bq_d)
        nc.sync.dma_start(out=bk_sb, in_=bk_d)
        nc.sync.dma_start(out=bp_sb, in_=bp_d)
        nc.sync.dma_start(out=bvb_sb, in_=bvb_d)

        qT_sb = qkv_p.tile([P, KC, N], BF16)
        kT_sb = qkv_p.tile([P, KC, N], BF16)
        vaug_sb = qkv_p.tile([P, MC, H, DK + 1], BF16)
        nc.vector.memset(vaug_sb[:, :, :, DK : DK + 1], 1.0)
        oT_sb = qkv_p.tile([P, KC, N], BF16)

        for _rep in range(reps):
            # ---- qkv projections (xT scoped; its SBUF is reused later) ----
            with tc.tile_pool(name="xtp", bufs=1) as xt_pool:
                xT_sb = xt_pool.tile([P, KC, N], BF16)
                for c in range(KC):
                    nc.sync.dma_start(out=xT_sb[:, c, :], in_=xT_v[:, c, :])
                # qT/kT: out[c*128+p, n] = sum_d w[d, c*128+p] * xT[d, n]
                for c in range(KC):
                    for dst_sb, w_sb, b_sb in (
                        (qT_sb, wq_sb, bq_sb),
                        (kT_sb, wk_sb, bk_sb),
                    ):
                        ps = psA.tile([P, N], F32, tag="psA", name="ps_qk")
                        for f in range(NF):
                            for k in range(KC):
                                nc.tensor.matmul(
                                    ps[:, f * FREE : (f + 1) * FREE],
                                    lhsT=w_sb[:, k, c * P : (c + 1) * P],
                                    rhs=xT_sb[:, k, f * FREE : (f + 1) * FREE],
                                    start=(k == 0),
                                    stop=(k == KC - 1),
                                )
                        nc.vector.tensor_scalar_add(
                            out=dst_sb[:, c, :], in0=ps, scalar1=b_sb[:, c : c + 1]
                        )
                # v natural: out[m, d] = sum_k x[m, k] w_v[k, d]
                for c in range(MC):
                    ps = psA.tile([P, D], F32, tag="psA", name="ps_v")
                    for f in range(D // FREE):
                        for k in range(KC):
                            nc.tensor.matmul(
                                ps[:, f * FREE : (f + 1) * FREE],
                                lhsT=xT_sb[:, k, c * P : (c + 1) * P],
                                rhs=wv_sb[:, k, f * FREE : (f + 1) * FREE],
                                start=(k == 0),
                                stop=(k == KC - 1),
                            )
                    nc.vector.tensor_add(
                        out=vaug_sb[:, c, :, 0:DK], in0=ps, in1=bvb_sb
                    )

            rep_ctx = ctx if reps == 1 else ExitStack()
            exp_p = rep_ctx.enter_context(tc.tile_pool(name="expp", bufs=2))
            misc_p = rep_ctx.enter_context(tc.tile_pool(name="misc", bufs=3))
            ystage_p = rep_ctx.enter_context(tc.tile_pool(name="ystage", bufs=2))

            # ---- attention (per pair of heads sharing a 128-row chunk) ----
            for c in range(KC):
                exA = exp_p.tile([P, MC, N], BF16, tag="exp", name="exA")
                exB = exp_p.tile([P, MC, N], BF16, tag="exp", name="exB")
                for j in range(MC):
                    sA = psA.tile([P, N], F32, tag="psA", name="sA")
                    sB = psA.tile([P, N], F32, tag="psA", name="sB")
                    for f in range(NF):
                        nc.tensor.matmul(
                            sA[:, f * FREE : (f + 1) * FREE],
                            lhsT=kT_sb[0:DK, c, j * P : (j + 1) * P],
                            rhs=qT_sb[0:DK, c, f * FREE : (f + 1) * FREE],
                            start=True,
                            stop=True,
                            tile_position=(0, 0),
                        )
                        nc.tensor.matmul(
                            sB[:, f * FREE : (f + 1) * FREE],
                            lhsT=kT_sb[DK:P, c, j * P : (j + 1) * P],
                            rhs=qT_sb[DK:P, c, f * FREE : (f + 1) * FREE],
                            start=True,
                            stop=True,
                            tile_position=(DK, 0),
                        )
                    nc.scalar.activation(
                        out=exA[:, j, :], in_=sA,
                        func=mybir.ActivationFunctionType.Exp,
                    )
                    nc.scalar.activation(
                        out=exB[:, j, :], in_=sB,
                        func=mybir.ActivationFunctionType.Exp,
                    )
                for hl, ex in ((0, exA), (1, exB)):
                    h = c * 2 + hl
                    for f in range(NF):
                        po = psB.tile([DK + 1, FREE], F32, tag="psB", name="po")
                        for j in range(MC):
                            nc.tensor.matmul(
                                po,
                                lhsT=vaug_sb[:, j, h, :],
                                rhs=ex[:, j, f * FREE : (f + 1) * FREE],
                                start=(j == 0),
                                stop=(j == MC - 1),
                            )
                        rc = misc_p.tile([1, FREE], F32, tag="recip", name="rc")
                        nc.vector.reciprocal(out=rc, in_=po[DK : DK + 1, :])
                        # broadcast 1/denom to DK partitions via a DRAM bounce
                        # (0-stride partition reads are only legal on DRAM APs)
                        dt_ = dram_p.tile([1, FREE], F32, tag="dbounce", name="dt_")
                        nc.sync.dma_start(out=dt_, in_=rc)
                        rcb = misc_p.tile([DK, FREE], F32, tag="rcb", name="rcb")
                        dt_bcast = bass.AP(
                            tensor=dt_.tensor,
                            offset=dt_.offset,
                            ap=[[0, DK]] + [list(d) for d in dt_.ap[1:]],
                        )
                        nc.sync.dma_start(out=rcb, in_=dt_bcast)
                        tmpo = misc_p.tile([DK, FREE], BF16, tag="tmpo", name="tmpo")
                        nc.vector.tensor_mul(out=tmpo, in0=po[0:DK, :], in1=rcb)
                        nc.sync.dma_start(
                            out=oT_sb[
                                hl * DK : (hl + 1) * DK, c,
                                f * FREE : (f + 1) * FREE,
                            ],
                            in_=tmpo,
                        )

            # ---- output projection ----
            for c in range(KC):
                ps = psA.tile([P, N], F32, tag="psA", name="ps_proj")
                for f in range(NF):
                    for k in range(KC):
                        nc.tensor.matmul(
                            ps[:, f * FREE : (f + 1) * FREE],
                            lhsT=wp_sb[:, k, c * P : (c + 1) * P],
                            rhs=oT_sb[:, k, f * FREE : (f + 1) * FREE],
                            start=(k == 0),
                            stop=(k == KC - 1),
                        )
                yst = ystage_p.tile([P, N], F32, tag="yst", name="yst")
                nc.vector.tensor_scalar_add(
                    out=yst, in0=ps, scalar1=bp_sb[:, c : c + 1]
                )
                nc.sync.dma_start(out=yT_v[:, c, :], in_=yst)

            if reps != 1:
                rep_ctx.close()

    nc.compile()
    return nc


def make_in_maps(x, w_qkv, b_qkv, w_proj, b_proj, N=1024, D=1024, H=16, DK=64):
    """Host-side prep: shard over batch, fold scale, transpose x, cast bf16."""
    bf = ml_dtypes.bfloat16
    KC = D // P
    scale = np.float32(1.0 / np.sqrt(DK))
    wq = np.ascontiguousarray((w_qkv[:, :D] * scale)).astype(bf)
    wk = np.ascontiguousarray(w_qkv[:, D : 2 * D]).astype(bf)
    wv = np.ascontiguousarray(w_qkv[:, 2 * D :]).astype(bf)
    wp = np.ascontiguousarray(w_proj).astype(bf)
    bq = np.ascontiguousarray((b_qkv[:D] * scale).reshape(KC, P).T).astype(np.float32)
    bk = np.ascontiguousarray(b_qkv[D : 2 * D].reshape(KC, P).T).astype(np.float32)
    bvb = np.ascontiguousarray(
        np.broadcast_to(b_qkv[2 * D :], (P, D))
    ).astype(bf)
    bp = np.ascontiguousarray(b_proj.reshape(KC, P).T).astype(np.float32)
    in_maps = []
    for b in range(x.shape[0]):
        xT = np.ascontiguousarray(x[b].T).astype(bf)
        in_maps.append(
            dict(xT=xT, wq=wq, wk=wk, wv=wv, wp=wp, bq=bq, bk=bk, bvb=bvb, bp=bp)
        )
    return in_maps


_module_cache = {}


def kernel(x, w_qkv, b_qkv, w_proj, b_proj):
    from concourse.bass_utils import run_bass_kernel_spmd

    x = np.asarray(x)
    B = x.shape[0]
    if "nc" not in _module_cache:
        _module_cache["nc"] = build_module()
    nc = _module_cache["nc"]
    in_maps = make_in_maps(
        x, np.asarray(w_qkv), np.asarray(b_qkv), np.asarray(w_proj), np.asarray(b_proj)
    )
    res = run_bass_kernel_spmd(nc, in_maps, core_ids=list(range(B)))
    out = np.stack([np.asarray(r["yT"]).T for r in res.results], axis=0)
    return np.ascontiguousarray(out.astype(np.float32))
